# revision 7
# baseline (speedup 1.0000x reference)
"""2-layer GCN message passing on a fixed-degree (K=5) KNN graph, 8-core SPMD.

out = x0 + x1 + x2,  x1 = w*A@x0,  x2 = w*A@x1,  (A@x)[n] = sum_k x[knn[n,k]]
with w = (K + 1e-7)^-1 computed in fp32 exactly as the reference.

Strategy (rows sharded 12500/core, padded to 12544 = 98*128 = T tiles):
 - gathers use the batched dma_gather custom instruction (<=1024 rows per
   instruction vs 128 for indirect DMA). Its int16 index limit is handled by
   a two-level scheme per 7-tile group (4480 edges):
     stage A: edges bucketed by uniform source-row RANGES (same bases on all
       cores; per-(group,range) capacity = max count over cores, baked at
       build); gathered relative to the range base into an SBUF tile
       (stream position i -> (i%128, i//128)), then streamed contiguously to
       a per-group DRAM scratch slab;
     stage B: dma_gathers from the slab (tile-local indices < 6k fit int16)
       restore k-major (p, k, t_l) order, so the 5-neighbor reduction is 4
       fully contiguous DVE adds.
 - the module is compiled per input inside kernel() (graph-dependent
   capacities are baked; index values stay runtime tensors).
 - host pre-scales the gather source (w*x0) so gather+sum yields x1 directly.
 - s1 = w*x1 on the Scalar engine; AllGather in 14 chunks overlapped with
   phase 1; phase 2 repeats the same machinery against the AllGather result.
"""

import os
import sys

import numpy as np


def _import_toolchain():
    try:
        import concourse.bass  # noqa: F401
        return
    except ImportError:
        pass
    for p in ("/opt/trn_rl_repo", os.path.expanduser("~/.axon_site/_ro/trn_rl_repo")):
        if os.path.isdir(p) and p not in sys.path:
            sys.path.insert(0, p)
    import concourse.bass  # noqa: F401


_import_toolchain()

from concourse import bacc, bass, mybir, tile  # noqa: E402
from concourse.bass_utils import run_bass_kernel_spmd  # noqa: E402

N = 100000
D = 128
K = 5
CORES = 8
RPC = N // CORES          # 12500 rows per core
T = 98                    # row-tiles of 128 per core (98*128 = 12544)
RPAD = T * 128            # 12544
G = 7                     # row-tiles per gather group
NG = T // G               # 14 groups
GC = G * D                # columns per group (896)
NE = G * K * 128          # edges per group (4480)
NB = 5                    # stage-B instructions per group
NBI = NE // NB            # 896 indices per stage-B instruction
NCH = 14                  # AllGather chunks (one per group)
TPC = T // NCH            # row-tiles per chunk (7)
NROWS2 = CORES * 128 * T  # s1full rows (100352)
MAXI = 1024               # dma_gather per-instruction index cap (HW ring)
NRANGE = 5                # stage-A source ranges (span must stay < 32768)
F32 = mybir.dt.float32
I16 = mybir.dt.int16


def _w_fp32() -> np.float32:
    rs = np.float32(5.0) + np.float32(1e-7)
    r = np.float32(np.float32(rs) ** np.float32(-0.5))
    return np.float32(r * r)


def _range_bounds(nsrc):
    """NRANGE uniform [lo, hi) ranges covering nsrc rows, span < 32768."""
    step = -(-nsrc // NRANGE)
    assert step < 32768
    return [(r * step, min(nsrc, (r + 1) * step)) for r in range(NRANGE)]


def _plan(glob_idx_all, nsrc):
    """Per (core, group): bucket edges by range; shared capacities over cores.

    glob_idx_all: [CORES, NG, NE] int64, edge -> source row, in output order
                  i = (k*G + t_l)*128 + p.
    Returns (caps[NG][NRANGE], per-core lists of (aidx, bidx) streams).
    """
    bounds = _range_bounds(nsrc)
    step = bounds[0][1] - bounds[0][0]
    rid = glob_idx_all // step                               # [C, NG, NE]
    counts = np.zeros((CORES, NG, NRANGE), np.int64)
    for r in range(NRANGE):
        counts[:, :, r] = (rid == r).sum(axis=2)
    caps = (-(-counts.max(axis=0) // 128) * 128)             # [NG, NRANGE]

    per_core = []
    for c in range(CORES):
        groups = []
        for gi in range(NG):
            gidx = glob_idx_all[c, gi]
            grid = rid[c, gi]
            order = np.argsort(grid, kind="stable")          # bucket by range
            rho = np.empty(NE, np.int64)
            aidx = []
            base_slot = 0
            s = 0
            for r in range(NRANGE):
                n = int(counts[c, gi, r])
                cap = int(caps[gi, r])
                sel = order[s:s + n]
                stream = np.zeros(cap, np.int16)
                rel = gidx[sel] - bounds[r][0]
                if n:
                    assert 0 <= rel.min() and rel.max() < 32768
                    stream[:n] = rel.astype(np.int16)
                aidx.append(stream)
                rho[sel] = base_slot * 128 + np.arange(n)
                base_slot += cap // 128
                s += n
            sa = base_slot
            brow = (rho % 128) * sa + rho // 128             # scratch DRAM row
            assert brow.max() < 32768
            groups.append((np.concatenate(aidx), brow.astype(np.int16)))
        per_core.append(groups)
    return caps, per_core


def _wrap16(a):
    """int16 stream -> [128, len/16] wrap-16 layout replicated per 16-block."""
    L = len(a)
    assert L % 16 == 0
    return np.ascontiguousarray(np.tile(a.reshape(L // 16, 16).T, (8, 1)))


def _build_nc(caps1, caps2):
    nc = bacc.Bacc("TRN2", target_bir_lowering=False, debug=False,
                   num_devices=CORES)
    w = float(_w_fp32())
    caps = {0: caps1, 1: caps2}
    sa = {ph: [int(caps[ph][gi].sum()) // 128 for gi in range(NG)]
          for ph in (0, 1)}

    x0s = nc.dram_tensor("x0s", [N, D], F32, kind="ExternalInput")       # w*x0
    x0m = nc.dram_tensor("x0m", [128, RPAD], F32, kind="ExternalInput")  # own rows
    yout = nc.dram_tensor("y", [128, RPAD], F32, kind="ExternalOutput")

    ia = [[nc.dram_tensor(f"ia{ph}_{gi}", [128, sa[ph][gi] * 8], I16,
                          kind="ExternalInput") for gi in range(NG)]
          for ph in (0, 1)]
    ib = [[nc.dram_tensor(f"ib{ph}_{gi}", [128, NE // 16], I16,
                          kind="ExternalInput") for gi in range(NG)]
          for ph in (0, 1)]
    scr = [[nc.dram_tensor(f"scr{ph}_{gi}", [128 * sa[ph][gi], D], F32)
            for gi in range(NG)] for ph in (0, 1)]

    x1loc = nc.dram_tensor("x1loc", [NCH, 128, TPC * D], F32)
    s1full = nc.dram_tensor("s1full", [NROWS2, D], F32, addr_space="Shared")
    s1ch = s1full.ap().rearrange("(ch x) d -> ch (x d)", ch=NCH)

    add = mybir.AluOpType.add
    copyf = mybir.ActivationFunctionType.Copy
    bounds = {0: _range_bounds(N), 1: _range_bounds(NROWS2)}

    with tile.TileContext(nc) as tc:
        with tc.tile_pool(name="pers", bufs=NG) as pers, \
             tc.tile_pool(name="idx", bufs=3) as idxp, \
             tc.tile_pool(name="ga", bufs=2) as gap, \
             tc.tile_pool(name="gb", bufs=2) as gbp, \
             tc.tile_pool(name="acc", bufs=2) as yp, \
             tc.tile_pool(name="io", bufs=3) as iop:

            partials = []

            def gather_sum(src, ph, gi, ytag):
                s_a = sa[ph][gi]
                ias = idxp.tile([128, s_a * 8], I16, tag="ia")
                nc.sync.dma_start(out=ias[:, :], in_=ia[ph][gi][:, :])
                ibs = idxp.tile([128, NE // 16], I16, tag="ib")
                nc.sync.dma_start(out=ibs[:, :], in_=ib[ph][gi][:, :])

                ga = gap.tile([128, s_a * D], F32, tag="ga")
                col = 0
                coff = 0
                for r in range(NRANGE):
                    lo, hi = bounds[ph][r]
                    cap = int(caps[ph][gi][r])
                    done = 0
                    while done < cap:
                        ni = min(MAXI, cap - done)
                        cc = ni // 128
                        nc.gpsimd.dma_gather(
                            ga[:, col * D:(col + cc) * D]
                            .rearrange("p (c d) -> p c d", d=D),
                            src[lo:hi, :],
                            ias[:, coff // 16:(coff + ni) // 16],
                            num_idxs=ni, num_idxs_reg=ni, elem_size=D)
                        col += cc
                        coff += ni
                        done += ni
                nc.sync.dma_start(out=scr[ph][gi].ap(), in_=ga[:, :])

                gb = gbp.tile([128, (NE // 128) * D], F32, tag="gb")
                gvb = gb[:, :].rearrange("p (s d) -> p s d", d=D)
                for m in range(NB):
                    so = m * (NBI // 128)
                    nc.gpsimd.dma_gather(
                        gvb[:, so:so + NBI // 128, :],
                        scr[ph][gi].ap(),
                        ibs[:, m * NBI // 16:(m + 1) * NBI // 16],
                        num_idxs=NBI, num_idxs_reg=NBI, elem_size=D)
                gk = gb[:, :].rearrange("p (k c) -> p k c", k=K)
                y = yp.tile([128, GC], F32, tag=ytag)
                nc.vector.tensor_tensor(out=y[:, :], in0=gk[:, 0], in1=gk[:, 1],
                                        op=add)
                for k in range(2, K):
                    nc.vector.tensor_tensor(out=y[:, :], in0=y[:, :],
                                            in1=gk[:, k], op=add)
                return y

            # ---- phase 1: x1 = gather-sum(w*x0); partial = x0 + x1;
            #      s1 = w*x1 -> AG input; AG per group.
            for gi in range(NG):
                cols = slice(gi * GC, (gi + 1) * GC)
                y = gather_sum(x0s, 0, gi, "y1")
                xt = iop.tile([128, GC], F32, tag="x0")
                nc.sync.dma_start(out=xt[:, :], in_=x0m[:, cols])
                part = pers.tile([128, GC], F32, tag="part")
                partials.append(part)
                nc.vector.tensor_tensor(out=part[:, :], in0=xt[:, :], in1=y[:, :],
                                        op=add)
                s1 = iop.tile([128, GC], F32, tag="s1")
                nc.scalar.activation(s1[:, :], y[:, :], copyf, scale=w)
                nc.sync.dma_start(out=x1loc[gi, :, :], in_=s1[:, :])
                nc.gpsimd.collective_compute(
                    "AllGather", mybir.AluOpType.bypass,
                    replica_groups=[list(range(CORES))],
                    ins=[x1loc[gi, :, :].opt()],
                    outs=[s1ch[gi].opt()],
                )

            # ---- phase 2: x2 = gather-sum(w*x1); out = partial + x2
            for gi in range(NG):
                cols = slice(gi * GC, (gi + 1) * GC)
                y2 = gather_sum(s1full, 1, gi, "y2")
                ot = yp.tile([128, GC], F32, tag="ot")
                nc.vector.tensor_tensor(out=ot[:, :], in0=partials[gi][:, :],
                                        in1=y2[:, :], op=add)
                nc.sync.dma_start(out=yout[:, cols], in_=ot[:, :])

    nc.finalize()
    return nc


_CACHE = {}


def _prep(item_rep, knn_ind):
    key = (item_rep.shape, knn_ind.shape, int(knn_ind[::997].sum()),
           int(knn_ind.sum()))
    if _CACHE.get("key") == key:
        return _CACHE["nc"], _CACHE["in_maps"]

    w = _w_fp32()
    x0s = np.ascontiguousarray(item_rep * w, dtype=np.float32)

    # layer-2 index remap: global row n -> row slot in s1full's layout
    # slot = ((ch*CORES + c) * 128 + p) * TPC + t_c  with t = ch*TPC + t_c
    c2 = knn_ind // RPC
    r2 = knn_ind - c2 * RPC
    t2 = r2 // 128
    p2 = r2 % 128
    ch2 = t2 // TPC
    tc2 = t2 - ch2 * TPC
    ind2_glob = ((ch2 * CORES + c2) * 128 + p2) * TPC + tc2

    def _edges_kmajor(ind_rows):
        """[RPAD, K] -> [NG, NE] in output order i=(k*G+t_l)*128+p."""
        a = ind_rows.reshape(NG, G, 128, K)           # [gi, t_l, p, k]
        return np.ascontiguousarray(
            a.transpose(0, 3, 1, 2).reshape(NG, NE)).astype(np.int64)

    def _pmajor(a):
        m = a.shape[1]
        return np.ascontiguousarray(
            a.reshape(T, 128, m).transpose(1, 0, 2).reshape(128, T * m))

    e1 = np.empty((CORES, NG, NE), np.int64)
    e2 = np.empty((CORES, NG, NE), np.int64)
    x0ms = []
    for c in range(CORES):
        rows = slice(c * RPC, (c + 1) * RPC)
        x0m = np.zeros((RPAD, D), np.float32)
        x0m[:RPC] = item_rep[rows]
        x0ms.append(_pmajor(x0m))
        i1 = np.zeros((RPAD, K), np.int64)
        i1[:RPC] = knn_ind[rows]
        i2 = np.zeros((RPAD, K), np.int64)
        i2[:RPC] = ind2_glob[rows]
        e1[c] = _edges_kmajor(i1)
        e2[c] = _edges_kmajor(i2)

    caps1, pc1 = _plan(e1, N)
    caps2, pc2 = _plan(e2, NROWS2)

    in_maps = []
    for c in range(CORES):
        im = {"x0s": x0s, "x0m": x0ms[c]}
        for ph, pc in ((0, pc1), (1, pc2)):
            for gi in range(NG):
                aidx, bidx = pc[c][gi]
                im[f"ia{ph}_{gi}"] = _wrap16(aidx)
                im[f"ib{ph}_{gi}"] = _wrap16(bidx)
        in_maps.append(im)

    nc = _build_nc(caps1, caps2)
    _CACHE.update(key=key, nc=nc, in_maps=in_maps)
    return nc, in_maps


def _unshard(outs):
    y = np.stack([outs[c]["y"] for c in range(CORES)])        # [8,128,12544]
    y = y.reshape(CORES, 128, T, D).transpose(0, 2, 1, 3)      # [8,98,128,128]
    return np.ascontiguousarray(y.reshape(CORES * RPAD, D)
                                .reshape(CORES, RPAD, D)[:, :RPC]
                                .reshape(N, D))


def kernel(item_rep, knn_ind, **_ignored):
    item_rep = np.asarray(item_rep, dtype=np.float32)
    knn_ind = np.asarray(knn_ind, dtype=np.int32)
    nc, in_maps = _prep(item_rep, knn_ind)
    res = run_bass_kernel_spmd(nc, in_maps, core_ids=list(range(CORES)))
    return _unshard(res.results)


# revision 8
# speedup vs baseline: 1.5968x; 1.5968x over previous
"""2-layer GCN message passing on a fixed-degree (K=5) KNN graph, 8-core SPMD.

out = x0 + x1 + x2,  x1 = w*A@x0,  x2 = w*A@x1,  (A@x)[n] = sum_k x[knn[n,k]]
with w = (K + 1e-7)^-1 computed in fp32 exactly as the reference.

Strategy (rows sharded 12500/core, padded to 12544 = 98*128 = T tiles):
 - host pre-scales the gather source (w*x0, fp32) so the layer-1 gather+sum
   yields x1 directly; per-core tensors are partition-major so regular DMAs
   are contiguous.
 - gathers use per-column indirect DMAs ([128,1] offset = 128 descriptors),
   the only indirect form this toolchain lowers correctly; indices are
   arranged k-major per 7-tile group so the 5-neighbor reduction is 4 fully
   contiguous DVE adds.
 - fp32 everywhere: fp16 gathers measured slower per-descriptor on HW.
 - s1 = w*x1 on the Scalar engine; AllGather in 14 chunks overlapped with
   phase 1 so only the last chunk's latency is exposed.
 - phase 2 gathers from the AllGather result; out = (x0+x1) + x2 in fp32.
"""

import os
import sys

import numpy as np


def _import_toolchain():
    try:
        import concourse.bass  # noqa: F401
        return
    except ImportError:
        pass
    for p in ("/opt/trn_rl_repo", os.path.expanduser("~/.axon_site/_ro/trn_rl_repo")):
        if os.path.isdir(p) and p not in sys.path:
            sys.path.insert(0, p)
    import concourse.bass  # noqa: F401


_import_toolchain()

from concourse import bacc, bass, mybir, tile  # noqa: E402
from concourse.bass_utils import run_bass_kernel_spmd  # noqa: E402

N = 100000
D = 128
K = 5
CORES = 8
RPC = N // CORES          # 12500 rows per core
T = 98                    # row-tiles of 128 per core (98*128 = 12544)
RPAD = T * 128            # 12544
G = 7                     # row-tiles per gather group
NG = T // G               # 14 groups
GC = G * D                # columns per group (896)
GK = G * K                # gather columns per group (35)
NCH = 14                  # AllGather chunks (divides NG)
GPC = NG // NCH           # groups per chunk (2)
TPC = T // NCH            # row-tiles per chunk (14)
F32 = mybir.dt.float32
F16 = mybir.dt.float16
I32 = mybir.dt.int32


def _w_fp32() -> np.float32:
    rs = np.float32(5.0) + np.float32(1e-7)
    r = np.float32(np.float32(rs) ** np.float32(-0.5))
    return np.float32(r * r)


def _build_nc():
    nc = bacc.Bacc("TRN2", target_bir_lowering=False, debug=False,
                   num_devices=CORES)
    w = float(_w_fp32())

    x0h = nc.dram_tensor("x0h", [N, D], F32, kind="ExternalInput")       # w*x0 (full)
    x0m = nc.dram_tensor("x0m", [128, RPAD], F32, kind="ExternalInput")  # own rows, p-major
    ind1 = nc.dram_tensor("ind1", [128, NG * GK], I32, kind="ExternalInput")
    ind2 = nc.dram_tensor("ind2", [128, NG * GK], I32, kind="ExternalInput")
    yout = nc.dram_tensor("y", [128, RPAD], F32, kind="ExternalOutput")

    x1loc = nc.dram_tensor("x1loc", [NCH, 128, TPC * D], F32)
    s1full = nc.dram_tensor("s1full", [CORES * 128 * T, D], F32, addr_space="Shared")
    s1ch = s1full.ap().rearrange("(ch x) d -> ch (x d)", ch=NCH)

    add = mybir.AluOpType.add
    copyf = mybir.ActivationFunctionType.Copy

    with tile.TileContext(nc) as tc:
        with tc.tile_pool(name="pers", bufs=NG) as pers, \
             tc.tile_pool(name="idx", bufs=2) as idxp, \
             tc.tile_pool(name="gat", bufs=4) as gp, \
             tc.tile_pool(name="acc", bufs=3) as yp, \
             tc.tile_pool(name="io", bufs=3) as iop:

            ind1_sb = idxp.tile([128, NG * GK], I32, tag="idx")
            nc.sync.dma_start(out=ind1_sb[:, :], in_=ind1[:, :])
            ind2_sb = idxp.tile([128, NG * GK], I32, tag="idx")
            nc.sync.dma_start(out=ind2_sb[:, :], in_=ind2[:, :])

            partials = []

            def gather_sum(src, ind_sb, gi, ytag):
                """GK per-column indirect DMAs (k-major) + 4 contiguous fp16
                DVE adds -> [128, GC]."""
                g = gp.tile([128, GK * D], F32, tag="g")
                gv = g[:, :].rearrange("p (j d) -> p j d", d=D)
                for j in range(GK):
                    col = gi * GK + j
                    nc.gpsimd.indirect_dma_start(
                        out=gv[:, j, :],
                        out_offset=None,
                        in_=src[:, :],
                        in_offset=bass.IndirectOffsetOnAxis(
                            ap=ind_sb[:, col:col + 1], axis=0),
                    )
                gk = g[:, :].rearrange("p (k c) -> p k c", k=K)
                y = yp.tile([128, GC], F32, tag=ytag)
                nc.vector.tensor_tensor(out=y[:, :], in0=gk[:, 0], in1=gk[:, 1],
                                        op=add)
                for k in range(2, K):
                    nc.vector.tensor_tensor(out=y[:, :], in0=y[:, :],
                                            in1=gk[:, k], op=add)
                return y

            # ---- phase 1: x1 = gather-sum(w*x0); partial = x0 + x1;
            #      s1 = w*x1 (fp16) -> AG input; AG per chunk of 2 groups.
            for gi in range(NG):
                cols = slice(gi * GC, (gi + 1) * GC)
                y = gather_sum(x0h, ind1_sb, gi, "y1")
                xt = iop.tile([128, GC], F32, tag="x0")
                nc.sync.dma_start(out=xt[:, :], in_=x0m[:, cols])
                part = pers.tile([128, GC], F32, tag="part")
                partials.append(part)
                nc.vector.tensor_tensor(out=part[:, :], in0=xt[:, :], in1=y[:, :],
                                        op=add)
                s1 = iop.tile([128, GC], F32, tag="s1")
                nc.scalar.activation(s1[:, :], y[:, :], copyf, scale=w)
                ch, off = gi // GPC, (gi % GPC) * GC
                nc.sync.dma_start(out=x1loc[ch, :, off:off + GC], in_=s1[:, :])
                if gi % GPC == GPC - 1:
                    nc.gpsimd.collective_compute(
                        "AllGather", mybir.AluOpType.bypass,
                        replica_groups=[list(range(CORES))],
                        ins=[x1loc[ch, :, :].opt()],
                        outs=[s1ch[ch].opt()],
                    )

            # ---- phase 2: x2 = gather-sum(w*x1); out = partial + x2
            for gi in range(NG):
                cols = slice(gi * GC, (gi + 1) * GC)
                y2 = gather_sum(s1full, ind2_sb, gi, "y2")
                ot = yp.tile([128, GC], F32, tag="ot")
                nc.vector.tensor_tensor(out=ot[:, :], in0=partials[gi][:, :],
                                        in1=y2[:, :], op=add)
                nc.sync.dma_start(out=yout[:, cols], in_=ot[:, :])

    nc.finalize()
    return nc


_NC_CACHE = {}


def _get_nc():
    if "nc" not in _NC_CACHE:
        _NC_CACHE["nc"] = _build_nc()
    return _NC_CACHE["nc"]


def _prep_inputs(item_rep, knn_ind):
    w = _w_fp32()
    x0h = np.ascontiguousarray(item_rep * w, dtype=np.float32)

    # layer-2 index remap: global row n -> row slot in s1full's layout
    # slot = ((ch*CORES + c) * 128 + p) * TPC + t_c   with t = ch*TPC + t_c
    c2 = knn_ind // RPC
    r2 = knn_ind - c2 * RPC
    t2 = r2 // 128
    p2 = r2 % 128
    ch2 = t2 // TPC
    tc2 = t2 - ch2 * TPC
    ind2_glob = (((ch2 * CORES + c2) * 128 + p2) * TPC + tc2).astype(np.int32)

    def _kmajor(ind_rows):
        """[RPAD, K] row-major -> [128, NG*GK] with col = gi*GK + k*G + t_l."""
        a = ind_rows.reshape(T, 128, K)               # [t, p, k]
        a = a.reshape(NG, G, 128, K)                  # [gi, t_l, p, k]
        a = a.transpose(2, 0, 3, 1)                   # [p, gi, k, t_l]
        return np.ascontiguousarray(a.reshape(128, NG * GK))

    def _pmajor(a):
        m = a.shape[1]
        return np.ascontiguousarray(
            a.reshape(T, 128, m).transpose(1, 0, 2).reshape(128, T * m))

    in_maps = []
    for c in range(CORES):
        rows = slice(c * RPC, (c + 1) * RPC)
        x0m = np.zeros((RPAD, D), np.float32)
        x0m[:RPC] = item_rep[rows]
        i1 = np.zeros((RPAD, K), np.int32)
        i1[:RPC] = knn_ind[rows]
        i2 = np.zeros((RPAD, K), np.int32)
        i2[:RPC] = ind2_glob[rows]
        in_maps.append({
            "x0h": x0h,
            "x0m": _pmajor(x0m),
            "ind1": _kmajor(i1),
            "ind2": _kmajor(i2),
        })
    return in_maps


def _unshard(outs):
    y = np.stack([outs[c]["y"] for c in range(CORES)])        # [8,128,12544]
    y = y.reshape(CORES, 128, T, D).transpose(0, 2, 1, 3)      # [8,98,128,128]
    return np.ascontiguousarray(y.reshape(CORES * RPAD, D)
                                .reshape(CORES, RPAD, D)[:, :RPC]
                                .reshape(N, D))


def kernel(item_rep, knn_ind, **_ignored):
    item_rep = np.asarray(item_rep, dtype=np.float32)
    knn_ind = np.asarray(knn_ind, dtype=np.int32)
    nc = _get_nc()
    in_maps = _prep_inputs(item_rep, knn_ind)
    res = run_bass_kernel_spmd(nc, in_maps, core_ids=list(range(CORES)))
    return _unshard(res.results)
